# revision 56
# baseline (speedup 1.0000x reference)
"""GAT multi-head attention (nn_GATMHAEfficient) on 8 Trainium2 NeuronCores.

Strategy (data-parallel over batch B=32 -> 4 graphs per core).

Score math is reformulated rank-1: since exp is monotonic and
leaky(t) = max(t, 0.2 t),

  exp(leaky(a_i + a_j)) = max(e^{a_i+a_j}, e^{0.2(a_i+a_j)})
                        = p_i * v_j * max(r_i, w_j)

with r_i = e^{0.8 a_i}, w_j = e^{-0.8 a_j}, v_j = e^{a_j}, p_i = e^{0.2 a_i}.
p_i is constant along the softmax axis and cancels in the normalization;
v_j folds into the aggregated values (g' = g * v_j, ones column -> v_j).
So the only O(N^2) per-head work is

  T_ji = max(r_i, w_j) * notm_ji          (j on partitions, i on free)

done as one tensor_scalar max (bf16, DVE 4x mode) plus one tensor_tensor
mask multiply (bf16, DVE 2x mode), load-balanced across engines per head:
  D: both ops on DVE
  P: single fused scalar_tensor_tensor on GPSIMD/Pool
  A: max(r,w) via two Relu activations on ACT (relu(r-w)+w), mask mult on DVE

Aggregation uses T as the matmul *stationary* operand and g' (17 cols,
bf16) as the moving operand, producing V[i-chunk, 17] directly in the
output orientation: no transposes and no PSUM->SBUF copies. Postproc is
one reciprocal + one fused (relu then scale) scalar_tensor_tensor per head.
"""

import json
import os
import tempfile

import numpy as np

# The neuron compile cache keys too weakly on the embedded bass module
# (identical shapes can hit a stale NEFF from a previous kernel version),
# so isolate this process's cache.
os.environ["NEURON_COMPILE_CACHE_URL"] = tempfile.mkdtemp(prefix="neff_cache_")

import concourse.bass as bass
import concourse.mybir as mybir
import concourse.tile as tile
from concourse.vector_clock import ScopedClock, VectorClock

F32 = mybir.dt.float32
F32R = mybir.dt.float32r
BF16 = mybir.dt.bfloat16
AF = mybir.ActivationFunctionType
ALU = mybir.AluOpType

B, N, NI, H, D = 32, 1024, 128, 8, 16
NCORES = 8
B_SH = B // NCORES          # graphs per core
C = N // 128                # j-chunks of 128
GRP = 4                     # j-chunks per mask-mult group
WCOLS = H * (D + 1) + 2 * H  # 152: per-head [W_h | 0] blocks, then W@Wal, W@War
GEXT = H * (D + 1)           # 136

# Engine assignment. On real TRN2 the Pool engine only supports
# TensorTensor mult/add (scalar_tensor_tensor and TT min/max fail the
# walrus engine check), so per chunk the max(r,w) runs on DVE ("D",
# tensor_scalar max, bf16 4x mode) or ACT ("A", two Relus), and the mask
# is a tensor_tensor multiply by the {0,1} gate per 4-chunk group on DVE
# ("V", bf16 2x mode) or Pool ("P").
# (maxpat, maskpat) per (b, h): maxpat is 8 chars D/A per chunk,
# maskpat V/P per 4-chunk group. A-chunks sit at group fronts and are
# spread thinly across heads so no head's aggregation waits long on ACT.
def _pat(b, h):
    if h == 0:
        return "DDDDDDDD", "VV"
    if h in (1, 2, 3, 4):
        return "AAADDDDD", "PV"
    if h == 5:
        return "AADDDDDD", "PV"
    if h == 6:
        return "AADDDDDD", "VV"
    return "AADDDDDD", "VV"  # h7


POSTPROC_LAG = 4  # heads between aggregation finish and its normalization

# ---------------------------------------------------------------------------
# Workarounds for this container's walrus build: it accepts at most ONE
# sync-wait per instruction, but Tile's sem-assignment (and its final drain)
# attach several. Split the excess onto dedicated single-wait EventSemaphore
# carrier instructions in the serialized BIR.


def _legalize_sync_waits(d, max_waits=1):
    for fn in d["functions"]:
        for bb in fn["blocks"]:
            new_insts = []
            for inst in bb["instructions"]:
                si = inst.get("sync_info") or {}
                w = si.get("on_wait") or []
                if len(w) > max_waits:
                    for k, we in enumerate(w[:-max_waits]):
                        new_insts.append(
                            {
                                "debug": inst.get("debug", 0),
                                "engine": inst["engine"],
                                "ins": [],
                                "outs": [],
                                "name": f"{inst['name']}_xw{k}",
                                "opcode": "EventSemaphore",
                                "sync_info": {"on_update": [], "on_wait": [we]},
                            }
                        )
                    si["on_wait"] = w[-max_waits:]
                new_insts.append(inst)
            bb["instructions"] = new_insts


def _wrap_to_json(nc):
    raw = nc.to_json_bytes

    def patched():
        d = json.loads(raw())
        _legalize_sync_waits(d)
        return json.dumps(d).encode()

    nc.to_json_bytes = patched


def _split_drain_and_barrier(self, tick_clock, wait_clock):
    # One drain per logical processor so each carries a single sem wait.
    gc = tick_clock.global_clock
    n = len(gc)
    for proc in range(n):
        t = gc[proc]
        if t > 0:
            dr = self.nc.sync.drain()
            pc = VectorClock([t if i == proc else 0 for i in range(n)])
            wait_clock.add_sem_waits(dr.ins, ScopedClock({None: pc}))
    self.nc.all_engine_barrier()
    popped = self.nc._tile_sem_poison_stack.pop()
    assert popped is self._sem_poison
    self.nc.clear_and_free_semaphores(list(self.sems.allocated().values()))
    self.nc.all_engine_barrier()


tile.TileContext._drain_and_barrier = _split_drain_and_barrier

# ---------------------------------------------------------------------------


def build_nc():
    nc = bass.Bass()
    hT = nc.dram_tensor("hT", [B_SH, NI, N], F32, kind="ExternalInput")
    notmT = nc.dram_tensor("notmT", [B_SH, N, N], BF16, kind="ExternalInput")
    wcat = nc.dram_tensor("wcat", [NI, WCOLS], F32, kind="ExternalInput")
    out = nc.dram_tensor("out", [B_SH, N, H * D], F32, kind="ExternalOutput")
    r_scr = nc.dram_tensor("r_scr", [B_SH, H, N], BF16)  # e^{0.8 a_i} rows

    from contextlib import ExitStack

    with ExitStack() as ctx:
        tc = ctx.enter_context(tile.TileContext(nc))
        const_p = ctx.enter_context(tc.tile_pool(name="const", bufs=1))
        hb_p = ctx.enter_context(tc.tile_pool(name="hb", bufs=B_SH))
        ge_p = ctx.enter_context(tc.tile_pool(name="ge", bufs=2))
        gp_p = ctx.enter_context(tc.tile_pool(name="gp", bufs=2))
        vw_p = ctx.enter_context(tc.tile_pool(name="vw", bufs=2))
        r8_p = ctx.enter_context(tc.tile_pool(name="r8", bufs=B_SH))
        bc_p = ctx.enter_context(tc.tile_pool(name="bc", bufs=12))
        sc_p = ctx.enter_context(tc.tile_pool(name="sc", bufs=6))
        scp_p = ctx.enter_context(tc.tile_pool(name="scp", bufs=5))
        tf_p = ctx.enter_context(tc.tile_pool(name="tf", bufs=2))
        rc_p = ctx.enter_context(tc.tile_pool(name="rc", bufs=2))
        ob_p = ctx.enter_context(tc.tile_pool(name="ob", bufs=2))
        nm_p = ctx.enter_context(tc.tile_pool(name="nm", bufs=2))
        xps_p = ctx.enter_context(tc.tile_pool(name="xps", bufs=2, space="PSUM"))
        xtps_p = ctx.enter_context(tc.tile_pool(name="xtps", bufs=1, space="PSUM"))
        vps_p = ctx.enter_context(tc.tile_pool(name="vps", bufs=5, space="PSUM"))

        wcat_s = const_p.tile([NI, WCOLS], F32)
        nc.sync.dma_start(out=wcat_s[:], in_=wcat[:])

        def _prep0():
            # Phase 0: for every graph, load h and run the whole
            # a_i -> r = e^{0.8 a_i} -> DRAM chain up front, so the
            # per-head broadcast DMAs never wait on mid-stream compute.
            # Fully interleaved per graph so r_scr[0] lands ASAP.
            hbTs = []
            for b in range(B_SH):
                hbT = hb_p.tile([NI, N], F32)
                for half in range(2):
                    src = bass.AP(
                        tensor=hT,
                        offset=b * NI * N + half * 512,
                        ap=[[N, NI], [1, 512]],
                    )
                    nc.sync.dma_start(
                        out=hbT[:, half * 512 : (half + 1) * 512], in_=src
                    )
                hbTs.append(hbT)
                r8 = r8_p.tile([H, N], BF16, tag="r8")
                for half in range(2):
                    sl = slice(half * 512, (half + 1) * 512)
                    XT_ps = xtps_p.tile([H, 512], F32, tag="xt")
                    nc.tensor.matmul(
                        XT_ps[:],
                        lhsT=wcat_s[:, GEXT : GEXT + H],
                        rhs=hbT[:, sl],
                        start=True,
                        stop=True,
                    )
                    nc.scalar.activation(
                        out=r8[:, sl], in_=XT_ps[:], func=AF.Exp, scale=0.8
                    )
                nc.sync.dma_start(out=r_scr[b], in_=r8[:])
            return hbTs

        hbTs = _prep0()

        def _prep(b):
            hbT = hbTs[b]
            notm = nm_p.tile([128, C, N], BF16)
            for k in range(4):  # split so broadcasts can interleave
                src = bass.AP(
                    tensor=notmT,
                    offset=(b * N + 2 * k * 128) * N,
                    ap=[[N, 128], [128 * N, 2], [1, N]],
                )
                nc.sync.dma_start(out=notm[:, 2 * k : 2 * k + 2, :], in_=src)

            # X = h_b @ Wcat per 128-row chunk: g cols (with zero slots),
            # a_i cols (136:144, unused here) and a_j cols (144:152).
            gext = ge_p.tile([128, C, WCOLS], F32)
            for c in range(C):
                X_ps = xps_p.tile([128, WCOLS], F32, tag="xv")
                nc.tensor.matmul(
                    X_ps[:],
                    lhsT=hbT[:, c * 128 : (c + 1) * 128],
                    rhs=wcat_s[:],
                    start=True,
                    stop=True,
                )
                nc.scalar.copy(out=gext[:, c, :], in_=X_ps[:])
            # ones column per head block -> becomes v_j after the g' scale
            ones_view = bass.AP(
                tensor=gext.tensor,
                offset=gext.offset + D,
                ap=[gext.ap[0], [WCOLS, C], [D + 1, H]],
            )
            nc.vector.memset(ones_view, 1.0)

            # per-j-partition vectors from the a_j columns of X
            aj_view = bass.AP(
                tensor=gext.tensor,
                offset=gext.offset + GEXT + H,
                ap=[gext.ap[0], [WCOLS, C], [1, H]],
            )
            vw = vw_p.tile([128, 3, C, H], F32, tag="vw")  # [v | w | -w]
            nc.scalar.activation(out=vw[:, 0], in_=aj_view, func=AF.Exp, scale=1.0)
            nc.scalar.activation(out=vw[:, 1], in_=aj_view, func=AF.Exp, scale=-0.8)
            nc.vector.tensor_scalar(
                out=vw[:, 2], in0=vw[:, 1], scalar1=-1.0, scalar2=None, op0=ALU.mult
            )

            # g' = g * v_j in bf16 (moving operand of the aggregation)
            gp = gp_p.tile([128, C, GEXT], BF16)
            for h in range(H):
                v_b = bass.AP(
                    tensor=vw.tensor,
                    offset=vw.offset + h,
                    ap=[vw.ap[0], [H, C], [0, D + 1]],
                )
                nc.vector.tensor_tensor(
                    out=gp[:, :, h * (D + 1) : (h + 1) * (D + 1)],
                    in0=gext[:, :, h * (D + 1) : (h + 1) * (D + 1)],
                    in1=v_b,
                    op=ALU.mult,
                )
            return notm, vw, gp

        def _emit_bcasts(b):
            tiles = []
            for h in range(H):
                bcR = bc_p.tile([128, N], BF16, tag="bc", name=f"bc_{b}_{h}")
                src = bass.AP(
                    tensor=r_scr,
                    offset=(b * H + h) * N,
                    ap=[[0, 128], [1, N]],
                )
                nc.sync.dma_start(out=bcR[:], in_=src)
                tiles.append(bcR)
            return tiles

        bcasts = {0: _emit_bcasts(0)}
        preps = {0: _prep(0)}

        def postproc(b, h, V_ps, ob):
            rc = rc_p.tile([128, C], F32)
            nc.vector.reciprocal(rc[:], V_ps[:, :, D])
            rc_b = bass.AP(
                tensor=rc.tensor,
                offset=rc.offset,
                ap=[rc.ap[0], [1, C], [0, D]],
            )
            nc.vector.scalar_tensor_tensor(
                out=ob[:, :, h * D : (h + 1) * D],
                in0=V_ps[:, :, 0:D],
                scalar=0.0,
                in1=rc_b,
                op0=ALU.max,
                op1=ALU.mult,
            )
            if h == H - 1:  # last head of graph b: result tile complete
                nc.sync.dma_start(
                    out=out[b].rearrange("(c p) d -> p c d", p=128), in_=ob[:]
                )

        pending = []
        for b in range(B_SH):
            notm, vw, gp = preps.pop(b)
            bcs = bcasts.pop(b)
            out_b = ob_p.tile([128, C, H * D], F32)

            for h in range(H):
                maxpat, maskpat = _pat(b, h)
                bcR = bcs[h]

                def w_col(c, k=1):
                    return bass.AP(
                        tensor=vw.tensor,
                        offset=vw.offset + (k * C + c) * H + h,
                        ap=[vw.ap[0], [1, 1]],
                    )

                def w_bcast(c):
                    return bass.AP(
                        tensor=vw.tensor,
                        offset=vw.offset + (C + c) * H + h,
                        ap=[vw.ap[0], [0, N]],
                    )

                V_ps = vps_p.tile([128, C, D + 1], F32, tag="vps")
                for g0 in range(0, C, GRP):
                    cs = list(range(g0, g0 + GRP))
                    tpool = sc_p if maskpat[g0 // GRP] == "V" else scp_p
                    t2 = tpool.tile([128, GRP, N], BF16, tag="sc", name=f"t_{b}_{h}_{g0}")
                    for i, c in enumerate(cs):
                        if maxpat[c] == "D":
                            nc.vector.tensor_scalar(
                                out=t2[:, i, :],
                                in0=bcR[:],
                                scalar1=w_col(c),
                                scalar2=None,
                                op0=ALU.max,
                            )
                        elif maxpat[c] == "P":  # max on Pool (w broadcast)
                            nc.gpsimd.tensor_tensor(
                                out=t2[:, i, :],
                                in0=bcR[:],
                                in1=w_bcast(c),
                                op=ALU.max,
                            )
                        else:  # "A"
                            tmpf = tf_p.tile([128, N], F32, tag="tf")
                            nc.scalar.activation(
                                out=tmpf[:],
                                in_=bcR[:],
                                func=AF.Relu,
                                bias=w_col(c, 2),  # -w
                                scale=1.0,
                            )
                            nc.scalar.activation(
                                out=t2[:, i, :],
                                in_=tmpf[:],
                                func=AF.Relu,
                                bias=w_col(c, 1),  # +w  (arg >= 0, so exact)
                                scale=1.0,
                            )
                    # mask: T = T0 * gate; DVE takes the whole group, Pool
                    # splits in two so its long ops pipeline with the
                    # producers and the aggregation
                    if maskpat[g0 // GRP] == "V":
                        nc.vector.tensor_tensor(
                            out=t2[:],
                            in0=t2[:],
                            in1=notm[:, g0 : g0 + GRP, :],
                            op=ALU.mult,
                        )
                    else:
                        for k in range(0, GRP, 2):
                            nc.gpsimd.tensor_tensor(
                                out=t2[:, k : k + 2, :],
                                in0=t2[:, k : k + 2, :],
                                in1=notm[:, g0 + k : g0 + k + 2, :],
                                op=ALU.mult,
                            )
                    # aggregation: T chunk stationary, g' (17 cols) moving
                    for i, c in enumerate(cs):
                        for ci in range(C):
                            # one accumulation group per head: start only
                            # on the bank's first matmul (a start marks the
                            # whole 2KB zero-region pending-zero, wiping any
                            # sibling region's partial sums)
                            nc.tensor.matmul(
                                V_ps[:, ci, :],
                                lhsT=t2[:, i, ci * 128 : (ci + 1) * 128],
                                rhs=gp[:, c, h * (D + 1) : (h + 1) * (D + 1)],
                                start=(c == 0 and ci == 0),
                                stop=(c == C - 1 and ci == C - 1),
                                skip_group_check=True,
                            )
                pending.append((b, h, V_ps, out_b))
                if len(pending) > POSTPROC_LAG:
                    postproc(*pending.pop(0))
                if h == 0 and b + 1 < B_SH:
                    preps[b + 1] = _prep(b + 1)
                    bcasts[b + 1] = _emit_bcasts(b + 1)

        while pending:
            postproc(*pending.pop(0))

    _wrap_to_json(nc)
    return nc


_NC_CACHE = None


def kernel(h, W, Wal, War, mask):
    global _NC_CACHE
    from concourse.bass_utils import run_bass_kernel_spmd

    h = np.asarray(h, dtype=np.float32)
    W = np.asarray(W, dtype=np.float32)
    Wal = np.asarray(Wal, dtype=np.float32)
    War = np.asarray(War, dtype=np.float32)
    import ml_dtypes

    # mask gate: 0 where masked, 1 where allowed (mask applied as multiply)
    notm_b16 = (~np.asarray(mask, dtype=bool)).astype(ml_dtypes.bfloat16)

    # Fold weights: wcat = [per-head (W_h | 0)] + [W@Wal] + [W@War]
    wcat = np.zeros((NI, WCOLS), dtype=np.float32)
    for hh in range(H):
        wcat[:, hh * (D + 1) : hh * (D + 1) + D] = W[hh]
        wcat[:, GEXT + hh] = W[hh] @ Wal[hh, :, 0]
        wcat[:, GEXT + H + hh] = W[hh] @ War[hh, :, 0]

    hT = np.ascontiguousarray(h.transpose(0, 2, 1))            # (B, I, N)
    notmT = np.ascontiguousarray(notm_b16.transpose(0, 2, 1))  # (B, j, i)

    if _NC_CACHE is None:
        _NC_CACHE = build_nc()
    nc = _NC_CACHE

    in_maps = []
    for core in range(NCORES):
        sl = slice(core * B_SH, (core + 1) * B_SH)
        in_maps.append(
            {
                "hT": np.ascontiguousarray(hT[sl]),
                "notmT": np.ascontiguousarray(notmT[sl]),
                "wcat": wcat,
            }
        )

    res = run_bass_kernel_spmd(nc, in_maps, list(range(NCORES)))
    out = np.concatenate([res.results[i]["out"] for i in range(NCORES)], axis=0)
    return out.astype(np.float32)
